# revision 22
# baseline (speedup 1.0000x reference)
"""Multi-head attention (B=4, S=2048, D=1024, H=16) on 8 Trainium2 cores.

Sharding: core c handles batch b = c//2 and head-group g = c%2 (8 heads,
512 features). Device program is identical on all cores (SPMD); the host
feeds each core its batch's activations (pre-transposed to [D, S]) and its
head-group's weight slices, sums the two partial output projections per
batch at the end, and adds the output bias on the host.

Device-side layout (per core):
  qT/kT: [512 f, S]  (f on partitions, chunked [128, 4, S]); head pair p
         (heads 2p, 2p+1) lives at partitions 0-63 / 64-127 of chunk p.
  v:     [S, 520]    (kj on partitions, per head 64 cols + a ones column)
  scores: one [128, 2*512] PSUM tile holds BOTH heads of a pair: the two
         K=64 score matmuls auto-derive tile_position (0,0)/(64,0) from
         their base partitions, so the PE can run them concurrently in
         disjoint row-strips.
  exp:   a single ACT instruction exps the packed [128, 1024] pair tile.
  PV:    out_aug[65, qi] = v_aug^T @ probsT per head (row 64 = softmax
         denominator via the ones column), accumulated over kj in PSUM.
  divide: two stages so the denominator-broadcast DMA round-trip (DRAM
         bounce) never stalls the DVE FIFO: stage1 (right after a block's
         last PV) copies the accumulators out of PSUM and launches the
         bounce; stage2 (a block later) does reciprocal + multiply + store.
  y:     yT[j, qi] partial = woT^T @ outT, f32 to DRAM (bias on host).

Pipeline shape (the critical part): with score tiles double-buffered and
PV emitted right after its exp, the engines serialize into a chain
exp(c-1) -> PV(c-1) -> scores(c) -> exp(c) (~1.28us/iter measured).
Breaking it needs the PE to run >1 iteration ahead of the ACT: score
tiles TRIPLE-buffered and PV delayed TWO iterations behind its exp
(measured 0.73us/iter). That takes all 8 PSUM banks (3x2 wide + 2 PV
accumulators), so the projections/output-projection run as serial phases
(borrowing wide-pool slots for their accumulations) instead of being
interleaved into the attention window.
"""

import numpy as np
import ml_dtypes

import concourse.bacc as bacc
import concourse.bass as bass
import concourse.mybir as mybir
import concourse.tile as tile

BF16 = mybir.dt.bfloat16
F32 = mybir.dt.float32

B, S, D, H = 4, 2048, 1024, 16
HD = 64
N_CORES = 8
F = D // 2   # features per core (8 heads x 64)
QB = 512     # query block (PSUM bank = 512 f32)
PV_DELAY = 2


def build_nc(s=S, d=D, f=F, num_devices=N_CORES, repeat=1):
    hpc = f // HD          # heads per core (8)
    npair = hpc // 2       # head pairs (4) == fc
    dc = d // 128          # contraction chunks for projections (8)
    fc = f // 128          # feature chunks (4)
    jc = d // 128          # output-feature chunks (8)
    kc = s // 128          # kj chunks for attention (16)
    nqb = s // QB          # query blocks (4)
    scale = 1.0 / np.sqrt(HD)
    assert f <= 512 and fc == npair

    nc = bacc.Bacc("TRN2", target_bir_lowering=False, debug=False,
                   num_devices=num_devices)

    xq = nc.dram_tensor("xq_t", [d, s], BF16, kind="ExternalInput").ap()
    xk = nc.dram_tensor("xk_t", [d, s], BF16, kind="ExternalInput").ap()
    xv = nc.dram_tensor("xv_t", [d, s], BF16, kind="ExternalInput").ap()
    wq = nc.dram_tensor("wq_t", [d, f], BF16, kind="ExternalInput").ap()
    wk = nc.dram_tensor("wk_t", [d, f], BF16, kind="ExternalInput").ap()
    wv = nc.dram_tensor("wv_t", [d, f], BF16, kind="ExternalInput").ap()
    wo = nc.dram_tensor("wo_t", [f, d], BF16, kind="ExternalInput").ap()
    y = nc.dram_tensor("y_t", [d, s], F32, kind="ExternalOutput").ap()

    with tile.TileContext(nc) as tc:
        with (
            tc.tile_pool(name="weights", bufs=1) as wpool,
            tc.tile_pool(name="store", bufs=1) as store,
            tc.tile_pool(name="xin", bufs=3) as xpool,
            tc.tile_pool(name="probs", bufs=PV_DELAY + 2) as ppool,
            tc.tile_pool(name="bcast", bufs=6) as bpool,
            tc.tile_pool(name="odiv", bufs=3) as opool,
            tc.tile_pool(name="ystage", bufs=3) as ypool,
            tc.tile_pool(name="ldram", bufs=4, space="DRAM") as dpool,
            tc.tile_pool(name="wide", bufs=3, space="PSUM") as wide,
            tc.tile_pool(name="acc", bufs=2, space="PSUM") as accp,
        ):
            # ---- persistent SBUF state ----
            wq_sb = wpool.tile([128, dc, f], BF16, tag="wq")
            wk_sb = wpool.tile([128, dc, f], BF16, tag="wk")
            wv_sb = wpool.tile([128, dc, f], BF16, tag="wv")
            wo_sb = wpool.tile([128, fc, d], BF16, tag="wo")
            # gpsimd queue: wv first (vproj starts ~3us in), then xk, wk, wq, wo
            nc.gpsimd.dma_start(out=wv_sb, in_=wv.rearrange("(c p) f -> p c f", p=128))

            kT_p = [store.tile([128, s], BF16, tag=f"kT{p}", name=f"kT{p}")
                    for p in range(npair)]
            qT_p = [store.tile([128, s], BF16, tag=f"qT{p}", name=f"qT{p}")
                    for p in range(npair)]
            v_p = [store.tile([128, kc, 130], BF16, tag=f"v{p}", name=f"v{p}")
                   for p in range(npair)]
            outT_p = [store.tile([128, s], BF16, tag=f"oT{p}", name=f"oT{p}")
                      for p in range(npair)]
            xk_sb = store.tile([128, dc, s], BF16, tag="xk")

            xkr = xk.rearrange("(c p) s -> p c s", p=128)
            h2 = dc // 2
            nc.gpsimd.dma_start(out=xk_sb[:, :h2], in_=xkr[:, :h2])
            nc.gpsimd.dma_start(out=xk_sb[:, h2:], in_=xkr[:, h2:])
            nc.gpsimd.dma_start(out=wk_sb, in_=wk.rearrange("(c p) f -> p c f", p=128))
            nc.gpsimd.dma_start(out=wq_sb, in_=wq.rearrange("(c p) f -> p c f", p=128))
            nc.gpsimd.dma_start(out=wo_sb, in_=wo.rearrange("(c p) j -> p c j", p=128))

            def psum_half():
                """[128, QB] psum accumulation target (half a wide slot)."""
                t = wide.tile([128, 2 * QB], F32, tag="wide", name="ph")
                return t[:, 0:QB]

            def kqproj(w_sb, x_rhs_of, dst, p, sl):
                ps = psum_half()
                for ci in range(dc):
                    nc.tensor.matmul(
                        ps, lhsT=w_sb[:, ci, p * 128:(p + 1) * 128],
                        rhs=x_rhs_of(ci),
                        start=(ci == 0), stop=(ci == dc - 1))
                nc.vector.tensor_copy(
                    out=dst[p][:, sl * QB:(sl + 1) * QB], in_=ps)

            def vproj_slice(sl, x_sb):
                for t in range(QB // 128):
                    ck = sl * (QB // 128) + t
                    ps = psum_half()
                    for ci in range(dc):
                        nc.tensor.matmul(
                            ps, lhsT=x_sb[:, ci, t * 128:(t + 1) * 128],
                            rhs=wv_sb[:, ci],
                            start=(ci == 0), stop=(ci == dc - 1))
                    for pp in range(npair):
                        nc.vector.tensor_copy(
                            out=v_p[pp][:, ck].rearrange(
                                "p (h x) -> p h x", x=65)[:, :, 0:64],
                            in_=ps[:, pp * 128:(pp + 1) * 128].rearrange(
                                "p (h x) -> p h x", x=64))

            def wo_group(qb, j):
                ps = psum_half()
                for fi in range(fc):
                    nc.tensor.matmul(
                        ps, lhsT=wo_sb[:, fi, j * 128:(j + 1) * 128],
                        rhs=outT_p[fi][:, qb * QB:(qb + 1) * QB],
                        start=(fi == 0), stop=(fi == fc - 1))
                ys = ypool.tile([128, QB], F32, tag="ys", name="ys")
                nc.vector.tensor_copy(out=ys, in_=ps)
                nc.sync.dma_start(
                    out=y[j * 128:(j + 1) * 128, qb * QB:(qb + 1) * QB],
                    in_=ys)

            # divide stages (DRAM-bounce broadcast, deferred reciprocal)
            divq = []

            def div_stage1(p, qb, oa, ob):
                ent = [p, qb]
                for oo in (oa, ob):
                    ls = bpool.tile([65, QB], F32, tag="ls", name="ls")
                    nc.vector.tensor_copy(out=ls, in_=oo)
                    ld = dpool.tile([1, QB], F32, tag="ld", name="ld")
                    nc.sync.dma_start(out=ld, in_=ls[64:65])
                    bc = bpool.tile([64, QB], F32, tag="bc", name="bc")
                    nc.scalar.dma_start(out=bc, in_=ld.to_broadcast([64, QB]))
                    ent += [ls, bc]
                divq.append(ent)

            def div_stage2():
                if not divq:
                    return
                p, qb, lsa, bca, lsb, bcb = divq.pop(0)
                qsl = slice(qb * QB, (qb + 1) * QB)
                for h, ls, bc in ((2 * p, lsa, bca), (2 * p + 1, lsb, bcb)):
                    hp = (h % 2) * 64
                    nc.vector.reciprocal_approx_fast(out=bc, in_=bc)
                    od = opool.tile([64, QB], BF16, tag="od", name="od")
                    nc.vector.tensor_mul(out=od, in0=ls[0:64], in1=bc)
                    nc.sync.dma_start(out=outT_p[p][hp:hp + 64, qsl], in_=od)

            # software pipeline: PV runs PV_DELAY c-iterations behind its exp
            # (across block boundaries) so the ACT streams exps back-to-back.
            pend = []

            def flush_pv():
                if not pend:
                    return
                p, qb, c, pr, oa, ob = pend.pop(0)
                nc.tensor.matmul(
                    oa, lhsT=v_p[p][:, c, 0:65],
                    rhs=pr[:, 0:QB],
                    start=(c == 0), stop=(c == kc - 1),
                    skip_group_check=True)
                nc.tensor.matmul(
                    ob, lhsT=v_p[p][:, c, 65:130],
                    rhs=pr[:, QB:2 * QB],
                    start=(c == 0), stop=(c == kc - 1),
                    skip_group_check=True)
                if c == kc - 1:
                    div_stage1(p, qb, oa, ob)
                    if len(divq) > 1:
                        div_stage2()

            def attn_block(p, qb):
                oa = accp.tile([65, QB], F32, tag="acc", name="oa")
                ob = accp.tile([65, QB], F32, tag="acc", name="ob")
                qsl = slice(qb * QB, (qb + 1) * QB)
                for c in range(kc):
                    sc = wide.tile([128, 2 * QB], F32, tag="wide", name="sc")
                    nc.tensor.matmul(
                        sc[:, 0:QB],
                        lhsT=kT_p[p][0:64, c * 128:(c + 1) * 128],
                        rhs=qT_p[p][0:64, qsl],
                        start=True, stop=True)
                    nc.tensor.matmul(
                        sc[:, QB:2 * QB],
                        lhsT=kT_p[p][64:128, c * 128:(c + 1) * 128],
                        rhs=qT_p[p][64:128, qsl],
                        start=True, stop=True)
                    pr = ppool.tile([128, 2 * QB], BF16, tag="pr", name="pr")
                    nc.scalar.activation(
                        out=pr, in_=sc,
                        func=mybir.ActivationFunctionType.Exp,
                        scale=float(scale))
                    if len(pend) >= PV_DELAY:
                        flush_pv()
                    pend.append((p, qb, c, pr, oa, ob))

            def body(_iv=None):
                # ones columns of v_aug (one strided memset per kj chunk)
                for pp in range(npair):
                    for c in range(kc):
                        nc.vector.memset(
                            v_p[pp][:, c].rearrange(
                                "p (h x) -> p h x", x=65)[:, :, 64:65],
                            1.0)

                # ---- projection phases (serial, PE-dense) ----
                xvr = xv.rearrange("(c p) s -> p c s", p=128)
                for sl in range(nqb):
                    x_sb = xpool.tile([128, dc, QB], BF16, tag="x", name="xv")
                    ssl = slice(sl * QB, (sl + 1) * QB)
                    nc.sync.dma_start(out=x_sb[:, :h2], in_=xvr[:, :h2, ssl])
                    nc.sync.dma_start(out=x_sb[:, h2:], in_=xvr[:, h2:, ssl])
                    vproj_slice(sl, x_sb)
                for p in range(npair):
                    for sl in range(nqb):
                        kqproj(wk_sb,
                               lambda ci, sl=sl: xk_sb[:, ci,
                                                       sl * QB:(sl + 1) * QB],
                               kT_p, p, sl)
                xqr = xq.rearrange("(c p) s -> p c s", p=128)
                for qb in range(nqb):
                    x_sb = xpool.tile([128, dc, QB], BF16, tag="x", name="xq")
                    qsl = slice(qb * QB, (qb + 1) * QB)
                    nc.sync.dma_start(out=x_sb[:, :h2], in_=xqr[:, :h2, qsl])
                    nc.sync.dma_start(out=x_sb[:, h2:], in_=xqr[:, h2:, qsl])
                    for p in range(npair):
                        kqproj(wq_sb, lambda ci, x_sb=x_sb: x_sb[:, ci],
                               qT_p, p, qb)

                # ---- attention window ----
                for p in range(npair):
                    for qb in range(nqb):
                        attn_block(p, qb)
                while pend:
                    flush_pv()
                while divq:
                    div_stage2()

                # ---- output projection ----
                for qb in range(nqb):
                    for j in range(jc):
                        wo_group(qb, j)

            if repeat == 1:
                body()
            else:
                with tc.For_i(0, repeat, 1) as iv:
                    body(iv)

    nc.compile()
    return nc


def make_core_inputs(query, key, value, wq, wk, wv, wo, bo):
    """Host-side sharding: per-core input dicts (bf16 casts + transposes)."""
    bf = ml_dtypes.bfloat16
    query, key, value = (np.asarray(t, np.float32) for t in (query, key, value))
    wq, wk, wv, wo = (np.asarray(t, np.float32) for t in (wq, wk, wv, wo))
    ins = []
    for c in range(N_CORES):
        b, g = c // 2, c % 2
        fs = slice(g * F, (g + 1) * F)
        ins.append({
            "xq_t": np.ascontiguousarray(query[b].astype(bf).T),
            "xk_t": np.ascontiguousarray(key[b].astype(bf).T),
            "xv_t": np.ascontiguousarray(value[b].astype(bf).T),
            "wq_t": np.ascontiguousarray(wq[fs, :].T.astype(bf)),
            "wk_t": np.ascontiguousarray(wk[fs, :].T.astype(bf)),
            "wv_t": np.ascontiguousarray(wv[fs, :].T.astype(bf)),
            "wo_t": np.ascontiguousarray(wo[:, fs].T.astype(bf)),
        })
    return ins


_NC_CACHE = None


def kernel(query, key, value, wq, wk, wv, wo, bo):
    global _NC_CACHE
    from concourse.bass_utils import run_bass_kernel_spmd

    if _NC_CACHE is None:
        _NC_CACHE = build_nc()
    ins = make_core_inputs(query, key, value, wq, wk, wv, wo, bo)
    res = run_bass_kernel_spmd(_NC_CACHE, ins, list(range(N_CORES)))
    bo32 = np.asarray(bo, np.float32)
    out = np.empty((B, S, D), np.float32)
    for b in range(B):
        out[b] = (res.results[2 * b]["y_t"] + res.results[2 * b + 1]["y_t"]).T
        out[b] += bo32
    return out
